# revision 12
# baseline (speedup 1.0000x reference)
"""Trainium2 Bass kernel for a 2-layer BiLSTM text classifier (v2).

Computation (matches the reference):
  e = emb[x]  ->  BiLSTM1 (return sequences)  ->  BiLSTM2 (return last state)
  -> softmax(h @ Wd + bd)

Sharding: pure data-parallel over batch across 8 cores (16 rows/core),
weights replicated, no collectives.

v2 design (cost-model driven):
  * xw = x@W+b is GEMMed directly INTO the PSUM chunk tile and the
    recurrent U-matmuls accumulate on top (start=False) -- no identity
    seed matmul, no PSUM->SBUF xw copy, no bias ACT pass.
    PSUM pending-zero semantics: exactly ONE start=True per 2KB bank
    (the first GEMM matmul), everything later start=False.
  * Both directions share one PSUM chunk tile -> ONE sigmoid and ONE
    tanh per (sub-chain, step) covering f+b, and the whole elementwise
    tail is fused across dirs at 2x width.
  * Backward-direction GEMMs read eT/seqT through reversed
    (negative-stride) APs -- no reversed copies, no per-t-slice matmuls.
  * Elementwise tail uses scalar_tensor_tensor (4x_2p eligible) on
    bf16 operands; the g-gate fixup (tanh(x)=2*sigmoid(2x)-1 with
    pre-scaled weights) runs on GPSIMD to offload DVE.
  * SUBS independent sub-chains (batch split) hide the ~1.5-2us serial
    step latency (PE -> ACT -> DVE -> ACT -> DVE -> PE).
"""

import numpy as np
import ml_dtypes

import concourse.bass as bass
import concourse.mybir as mybir
import concourse.tile as tile
from concourse import bacc
from concourse.bass_utils import run_bass_kernel_spmd
from concourse.masks import make_identity

# Problem dims (hardcoded per spec)
B, T, V, D, H, C = 128, 512, 50000, 128, 256, 10
NCORES = 8
BL = B // NCORES          # 16 batch rows per core
G = 4 * H                 # 1024 gate width
NM = G // 128             # 8 gate m-tiles
CH = 8                    # scan steps per PSUM chunk
NCH = T // CH             # 64 chunks
NTOK = T * BL             # 8192 tokens per core, time-major col = t*BL + j
GCH = NTOK // 128         # 64 embedding gather chunks

import os
SUBS = int(os.environ.get("SUBS", "2"))  # batch sub-chains per core
DOUBLE_ROW = os.environ.get("DR", "1") == "1"  # 2 k-tiles per matmul
SB = BL // SUBS           # batch rows per sub-chain

F32 = mybir.dt.float32
BF16 = mybir.dt.bfloat16
I32 = mybir.dt.int32
BF = ml_dtypes.bfloat16
AF = mybir.ActivationFunctionType
ALU = mybir.AluOpType

C_BF16 = True             # keep cell state c in bf16 (2x DVE mode)
GG_ON_POOL = True         # g-gate fixup on GPSIMD instead of DVE

TRACE = False
LAST_RESULTS = None

# Keras gate order i,f,g,o (H each) -> i,f,o,g so sigmoid gates contiguous
# m-tiles: 0,1=i  2,3=f  4,5=o  6,7=g(tanh via 2*sigmoid(2x)-1, pre-scaled)
_PERM = np.concatenate(
    [np.arange(0, 2 * H), np.arange(3 * H, 4 * H), np.arange(2 * H, 3 * H)]
)


def _pack_k(w, kt, dt=None):
    """[kt*128, G] -> [128, kt, G] k-tile packing (partition-major)."""
    return np.ascontiguousarray(
        w.reshape(kt, 128, w.shape[1]).transpose(1, 0, 2)
    ).astype(BF if dt is None else dt)


def _prep_weights(inputs):
    f32 = np.float32
    out = {}
    out["emb"] = np.ascontiguousarray(np.asarray(inputs["emb"], f32))
    fp8 = ml_dtypes.float8_e4m3
    for nm, kt, dt in [("U1f", 2, fp8), ("U1b", 2, fp8),
                       ("U2f", 2, fp8), ("U2b", 2, fp8),
                       ("W1f", 1, None), ("W1b", 1, None),
                       ("W2f", 4, fp8), ("W2b", 4, fp8)]:
        if not DOUBLE_ROW:
            dt = None
        w = np.asarray(inputs[nm], f32)[:, _PERM].copy()
        w[:, 3 * H:] *= 2.0
        out[nm.lower()] = _pack_k(w, kt, dt)
    for lay in (1, 2):
        bs = []
        for dn in ("f", "b"):
            b_ = np.asarray(inputs[f"b{lay}{dn}"], f32)[_PERM].copy()
            b_[3 * H:] *= 2.0
            bs.append(b_)
        out[f"bias{lay}"] = np.concatenate(bs).reshape(1, 2 * G).astype(BF)
    wd = np.asarray(inputs["Wd"], f32)  # [2H, C]
    out["wd"] = np.ascontiguousarray(
        wd.reshape(4, 128, C).transpose(1, 0, 2)
    ).astype(BF)
    out["bd"] = np.asarray(inputs["bd"], f32).reshape(1, C).astype(BF)
    return out


def _rev(ap, dim):
    """Reverse one free dim of an AP (negative stride view)."""
    ap = ap.copy()
    st, cnt = ap.ap[dim]
    ap.ap[dim] = [-st, cnt]
    ap.offset = ap.offset + st * (cnt - 1)
    return ap


def _build(with_bias):
    nc = bacc.Bacc("TRN2", target_bir_lowering=False, debug=False,
                   num_devices=NCORES)

    emb_d = nc.dram_tensor("emb", [V, D], F32, kind="ExternalInput")
    xidx_d = nc.dram_tensor("xidx", [128, GCH], I32, kind="ExternalInput")
    wdram = {}
    RDT = mybir.dt.float8e4 if DOUBLE_ROW else BF16
    for nm in ["u1f", "u1b", "u2f", "u2b"]:
        wdram[nm] = nc.dram_tensor(nm, [128, 2, G], RDT, kind="ExternalInput")
    for nm in ["w1f", "w1b"]:
        wdram[nm] = nc.dram_tensor(nm, [128, 1, G], BF16, kind="ExternalInput")
    for nm in ["w2f", "w2b"]:
        wdram[nm] = nc.dram_tensor(nm, [128, 4, G], RDT, kind="ExternalInput")
    for nm in ["bias1", "bias2"]:
        wdram[nm] = nc.dram_tensor(nm, [1, 2 * G], BF16, kind="ExternalInput")
    wdram["wd"] = nc.dram_tensor("wd", [128, 4, C], BF16, kind="ExternalInput")
    wdram["bd"] = nc.dram_tensor("bd", [1, C], BF16, kind="ExternalInput")
    out_d = nc.dram_tensor("out", [BL, C], F32, kind="ExternalOutput")

    CDT = BF16 if C_BF16 else F32

    with tile.TileContext(nc) as tc, \
         tc.tile_pool(name="const", bufs=1) as const, \
         tc.tile_pool(name="work", bufs=2) as work, \
         tc.tile_pool(name="pszz", bufs=2, space="PSUM") as pszz:

        sb = {}
        for nm, th in wdram.items():
            t_ = const.tile(list(th.shape), th.dtype, name=f"sb_{nm}",
                            tag=f"sb_{nm}")
            nc.sync.dma_start(out=t_[:], in_=th[:])
            sb[nm] = t_
        xidx = const.tile([128, GCH], I32, name="xidx_s", tag="xidx_s")
        nc.sync.dma_start(out=xidx[:], in_=xidx_d[:])

        ident = const.tile([128, 128], F32, name="ident", tag="ident")
        make_identity(nc, ident[:])
        zero_h = const.tile([128, 2, SB],
                            mybir.dt.float8e4 if DOUBLE_ROW else BF16,
                            name="zero_h", tag="zero_h")
        nc.vector.memset(zero_h[:], 0.0)
        ones_r = const.tile([1, CH, SB], BF16, name="ones_r", tag="ones_r")
        nc.vector.memset(ones_r[:], 1.0)
        ones_b = const.tile([1, BL], BF16, name="ones_b", tag="ones_b")
        nc.vector.memset(ones_b[:], 1.0)

        HDT = mybir.dt.float8e4 if DOUBLE_ROW else BF16
        eT = const.tile([128, NTOK], BF16, name="eT", tag="eT")
        seqT = const.tile([128, 4, NTOK], HDT, name="seqT", tag="seqT")
        c_st = [const.tile([128, 2, 2, SB], CDT, name=f"c{s}", tag=f"c{s}")
                for s in range(SUBS)]

        def zz_tile(s):
            # [d, m, t, b]; per-dir slice = 8m*CH*SB*4B = 2KB = one bank
            return pszz.tile([128, 2, NM, CH, SB], F32, name=f"zz{s}",
                             tag=f"zz{s}")

        # ---- stage A: embedding gather + transpose -> eT bf16 ----
        for ch in range(GCH):
            erows = work.tile([128, D], F32, name="erows", tag="erows", bufs=3)
            nc.gpsimd.indirect_dma_start(
                out=erows[:], out_offset=None, in_=emb_d[:],
                in_offset=bass.IndirectOffsetOnAxis(ap=xidx[:, ch:ch + 1],
                                                    axis=0))
            tpz = zz_tile(0)
            tp = tpz[:].rearrange("p a b c d -> p (a b c d)")[:, 0:128]
            nc.tensor.transpose(out=tp, in_=erows[:], identity=ident[:])
            nc.vector.tensor_copy(out=eT[:, ch * 128:(ch + 1) * 128], in_=tp)

        eT3 = eT[:].rearrange("p (t j) -> p t j", t=T)

        # ---- the scan ----
        h2 = [None] * SUBS          # layer-2 recurrent h tiles per sub

        def gemm_chunk(lay, cc, s, zz):
            """xw GEMM for chunk cc, sub s, both dirs, into PSUM zz."""
            c0, c1 = cc * CH, (cc + 1) * CH
            if lay == 1:
                rf = eT3[:, c0:c1, s * SB:(s + 1) * SB]
                rb = _rev(eT3[:, T - c1:T - c0, s * SB:(s + 1) * SB], 1)
                nk, wf, wb = 1, sb["w1f"], sb["w1b"]
            else:
                nk, wf, wb = 4, sb["w2f"], sb["w2b"]
            dr = False  # interp DoubleRow needs 2D-per-k rhs; rec only
            for d, w_ in ((0, wf), (1, wb)):
                for m in range(NM):
                    # start=True on the first matmul touching each 2KB
                    # PSUM bank (pending-zero is bank-granular)
                    st0 = (m * CH * SB * 4) % 2048 == 0
                    if dr:
                        for k2 in range(nk // 2):
                            sq = seqT[:, 2 * k2:2 * k2 + 2, :].rearrange(
                                "p a (t j) -> p a t j", t=T)
                            if d == 0:
                                rhs = sq[:, :, c0:c1, s * SB:(s + 1) * SB]
                            else:
                                rhs = _rev(sq[:, :, T - c1:T - c0,
                                              s * SB:(s + 1) * SB], 2)
                            nc.tensor.matmul(
                                zz[:, d, m, :, :],
                                lhsT=w_[:, 2 * k2:2 * k2 + 2,
                                        m * 128:(m + 1) * 128],
                                rhs=rhs, start=(k2 == 0 and st0), stop=False,
                                skip_group_check=True,
                                perf_mode=mybir.MatmulPerfMode.DoubleRow)
                        continue
                    for k in range(nk):
                        if lay == 1:
                            rhs = rf if d == 0 else rb
                        else:
                            sq = seqT[:, k, :].rearrange("p (t j) -> p t j",
                                                         t=T)
                            if d == 0:
                                rhs = sq[:, c0:c1, s * SB:(s + 1) * SB]
                            else:
                                rhs = _rev(sq[:, T - c1:T - c0,
                                              s * SB:(s + 1) * SB], 1)
                        nc.tensor.matmul(
                            zz[:, d, m, :, :],
                            lhsT=w_[:, k, m * 128:(m + 1) * 128], rhs=rhs,
                            start=(k == 0 and st0),
                            stop=False, skip_group_check=True)
                if with_bias:
                    for m in range(NM):
                        nc.tensor.matmul(
                            zz[:, d, m, :, :],
                            lhsT=sb[f"bias{lay}"][:, (d * NM + m) * 128:
                                                  (d * NM + m + 1) * 128],
                            rhs=ones_r[:], start=False, stop=False,
                            skip_group_check=True)

        def step(lay, cc, j, s, zz):
            t_f = cc * CH + j            # forward time index
            t_b = T - 1 - t_f            # backward time index
            u = sb[f"u{lay}f"], sb[f"u{lay}b"]
            # recurrent h inputs per dir: [128, 2(k), SB] APs
            if lay == 1:
                if t_f == 0:
                    hp2 = [zero_h[:], zero_h[:]]
                else:
                    cf = (t_f - 1) * BL + s * SB
                    cb = (t_b + 1) * BL + s * SB
                    hp2 = [seqT[:, 0:2, cf:cf + SB],
                           seqT[:, 2:4, cb:cb + SB]]
            else:
                if h2[s] is None:
                    hp2 = [zero_h[:], zero_h[:]]
                else:
                    hh = h2[s]
                    hp2 = [hh[:, 0], hh[:, 1]]
            for d in range(2):
                for m in range(NM):
                    if DOUBLE_ROW:
                        nc.tensor.matmul(
                            zz[:, d, m, j, :],
                            lhsT=u[d][:, :, m * 128:(m + 1) * 128],
                            rhs=hp2[d], start=False,
                            stop=(j == CH - 1 and m == NM - 1),
                            skip_group_check=True,
                            perf_mode=mybir.MatmulPerfMode.DoubleRow)
                        continue
                    for k in range(2):
                        nc.tensor.matmul(
                            zz[:, d, m, j, :],
                            lhsT=u[d][:, k, m * 128:(m + 1) * 128],
                            rhs=hp2[d][:, k, :], start=False,
                            stop=(j == CH - 1 and m == NM - 1 and k == 1),
                            skip_group_check=True)
            g = work.tile([128, 2, NM, SB], BF16, name=f"g{s}", tag=f"g{s}",
                          bufs=3)
            nc.scalar.activation(out=g[:], in_=zz[:, :, :, j, :],
                                 func=AF.Sigmoid)
            c = c_st[s]
            t2 = work.tile([128, 2, 2, SB], CDT, name=f"t2_{s}", tag=f"t2_{s}",
                           bufs=3)
            nc.vector.scalar_tensor_tensor(
                out=t2[:], in0=c[:], scalar=1.0, in1=g[:, :, 2:4, :],
                op0=ALU.mult, op1=ALU.mult)
            gg = work.tile([128, 2, 2, SB], BF16, name=f"gg{s}", tag=f"gg{s}",
                           bufs=3)
            eng = nc.gpsimd if GG_ON_POOL else nc.vector
            eng.tensor_scalar(out=gg[:], in0=g[:, :, 6:8, :],
                              scalar1=2.0, scalar2=1.0,
                              op0=ALU.mult, op1=ALU.subtract)
            t1 = work.tile([128, 2, 2, SB], CDT, name=f"t1_{s}", tag=f"t1_{s}",
                           bufs=3)
            nc.vector.scalar_tensor_tensor(
                out=t1[:], in0=gg[:], scalar=1.0, in1=g[:, :, 0:2, :],
                op0=ALU.mult, op1=ALU.mult)
            nc.vector.scalar_tensor_tensor(
                out=c[:], in0=t1[:], scalar=0.0, in1=t2[:],
                op0=ALU.add, op1=ALU.add)
            th = work.tile([128, 2, 2, SB], BF16, name=f"th{s}", tag=f"th{s}",
                           bufs=3)
            nc.scalar.activation(out=th[:], in_=c[:], func=AF.Tanh)
            if lay == 1:
                for d, tt in ((0, t_f), (1, t_b)):
                    nc.vector.scalar_tensor_tensor(
                        out=seqT[:, 2 * d:2 * d + 2,
                                 tt * BL + s * SB:tt * BL + s * SB + SB],
                        in0=g[:, d, 4:6, :], scalar=1.0, in1=th[:, d],
                        op0=ALU.mult, op1=ALU.mult)
            else:
                hn = work.tile([128, 2, 2, SB], HDT, name=f"h2_{s}",
                               tag=f"h2_{s}", bufs=3)
                nc.vector.scalar_tensor_tensor(
                    out=hn[:], in0=g[:, :, 4:6, :], scalar=1.0, in1=th[:],
                    op0=ALU.mult, op1=ALU.mult)
                h2[s] = hn

        for lay in (1, 2):
            for s in range(SUBS):
                nc.vector.memset(c_st[s][:], 0.0)
                h2[s] = None
            for cc in range(NCH):
                zzs = [zz_tile(s) for s in range(SUBS)]
                for s in range(SUBS):
                    gemm_chunk(lay, cc, s, zzs[s])
                for j in range(CH):
                    for s in range(SUBS):
                        step(lay, cc, j, s, zzs[s])

        # ---- dense + softmax ----
        pz = zz_tile(0)
        ps = pz[:].rearrange("p a b c d -> p (a b c d)")[0:BL, 0:C]
        hT = const.tile([128, 2, 2, BL], BF16, name="hT", tag="hT")
        for s in range(SUBS):
            nc.vector.tensor_copy(out=hT[:, :, :, s * SB:(s + 1) * SB],
                                  in_=h2[s][:])
        for i, (d, k) in enumerate([(0, 0), (0, 1), (1, 0), (1, 1)]):
            nc.tensor.matmul(
                ps, lhsT=hT[:, d, k, :],
                rhs=sb["wd"][:, i, :], start=(i == 0),
                stop=False, skip_group_check=True)
        nc.tensor.matmul(ps[:, :], lhsT=ones_b[:], rhs=sb["bd"][:],
                         start=False, stop=True, skip_group_check=True)
        mx = work.tile([BL, 1], F32, name="mx", tag="mx")
        nc.vector.reduce_max(out=mx[:], in_=ps, axis=mybir.AxisListType.X)
        mxn = work.tile([BL, 1], F32, name="mxn", tag="mxn")
        nc.vector.tensor_scalar_mul(mxn[:], mx[:], -1.0)
        ex = work.tile([BL, C], F32, name="ex", tag="ex")
        sm = work.tile([BL, 1], F32, name="sm", tag="sm")
        nc.scalar.activation(out=ex[:], in_=ps, func=AF.Exp,
                             bias=mxn[:, 0:1], scale=1.0, accum_out=sm[:])
        rs = work.tile([BL, 1], F32, name="rs", tag="rs")
        nc.vector.reciprocal(rs[:], sm[:])
        osm = work.tile([BL, C], F32, name="osm", tag="osm")
        nc.vector.tensor_scalar_mul(osm[:], ex[:], rs[:, 0:1])
        nc.sync.dma_start(out=out_d[:], in_=osm[:])

    nc.compile()
    return nc


_CACHE = {}


def make_in_maps(inputs):
    w = _prep_weights(inputs)
    x = np.asarray(inputs["x"], np.int32)  # [B, T]
    in_maps = []
    for core in range(NCORES):
        xc = x[core * BL:(core + 1) * BL]            # [BL, T]
        tm = np.ascontiguousarray(xc.T).reshape(-1)  # time-major [T*BL]
        xi = np.ascontiguousarray(tm.reshape(GCH, 128).T).astype(np.int32)
        m = {"xidx": xi, "emb": w["emb"]}
        for nm in ["u1f", "u1b", "u2f", "u2b", "w2f", "w2b",
                   "bias1", "bias2", "wd", "bd"]:
            m[nm] = w[nm]
        for nm in ["w1f", "w1b"]:
            m[nm] = w[nm].reshape(128, 1, G)
        in_maps.append(m)
    return in_maps


def _has_bias(inputs):
    return any(np.any(np.asarray(inputs[nm]))
               for nm in ["b1f", "b1b", "b2f", "b2b"])


def get_nc(with_bias=True):
    key = ("nc", bool(with_bias))
    if key not in _CACHE:
        _CACHE[key] = _build(bool(with_bias))
    return _CACHE[key]


def kernel(**inputs):
    global LAST_RESULTS
    nc = get_nc(_has_bias(inputs))
    in_maps = make_in_maps(inputs)
    res = run_bass_kernel_spmd(nc, in_maps, core_ids=list(range(NCORES)),
                               trace=TRACE)
    LAST_RESULTS = res
    return np.concatenate([r["out"] for r in res.results], axis=0)


# revision 16
# speedup vs baseline: 1.7133x; 1.7133x over previous
"""Trainium2 Bass kernel for a 2-layer BiLSTM text classifier.

Computation (matches the reference):
  e = emb[x]  ->  BiLSTM1 (return sequences)  ->  BiLSTM2 (return last state)
  -> softmax(h @ Wd + bd)

Sharding: pure data-parallel over batch across 8 cores (16 rows/core),
weights replicated, no collectives.  Each core runs all 4 scans; the fwd
and bwd directions of a layer are interleaved as two independent
dependency chains so PE/ACT/DVE stay busy.

Layout: "gates on partitions".  z.T for one step lives in one PSUM bank
as [128, 8*16] (8 gate m-tiles of 128 rows x 16 batch).  Recurrent
matmuls keep U tiles stationary ([128,128] bf16) and stream h
([128,16] bf16).  The input projection x@W+b is precomputed in 32-step
chunks as efficient N=512 matmuls into a 2-deep SBUF window (one m-slice
emitted per scan step to avoid FIFO bursts) and added to z with one DVE
op per step.  Gate order is permuted to (i,f,o,g) and the g-gate weights
are pre-scaled by 2 so ONE sigmoid serves all gates
(tanh(x) = 2*sigmoid(2x)-1, fixed up on DVE).  Measured wall cost is
~88 ns per instruction regardless of data size, so the design minimizes
instruction count: layer 1's h history is read directly from seqT
(single DVE write/step), and each direction-step is 16 MM + 2 ACT +
6 DVE instructions.
"""

import os

import numpy as np
import ml_dtypes

import concourse.bass as bass
import concourse.mybir as mybir
import concourse.tile as tile
from concourse import bacc
from concourse.bass_utils import run_bass_kernel_spmd
from concourse.masks import make_identity

# Problem dims (hardcoded per spec)
B, T, V, D, H, C = 128, 512, 50000, 128, 256, 10
NCORES = 8
BL = B // NCORES          # 16 batch rows per core
G = 4 * H                 # 1024 gate width
NM = G // 128             # 8 gate m-tiles
CHUNK = 32                # scan steps per xW chunk
NCH = T // CHUNK          # 16 chunks
NTOK = T * BL             # 8192 tokens per core, time-major (col = t*BL + j)
GCH = NTOK // 128         # 64 embedding gather chunks

F32 = mybir.dt.float32
BF16 = mybir.dt.bfloat16
I32 = mybir.dt.int32
BF = ml_dtypes.bfloat16
AF = mybir.ActivationFunctionType

# Recurrent-matmul operand dtype.  fp8_e3m4 (range +-15.9, ~1.6% step)
# halves the PE weight-load traffic vs bf16-FWL for the per-step U reload.
RECUR_FP8 = os.environ.get("RECUR_FP8", "0") == "1"
RDT = mybir.dt.float8e3 if RECUR_FP8 else BF16
RNP = ml_dtypes.float8_e3m4 if RECUR_FP8 else BF

# Timing-ablation probe ("" = full kernel, "mm" = matmuls only).
PROBE = os.environ.get("PROBE", "")
HALVES = int(os.environ.get("HALVES", "2"))  # sub-chains per direction
HB = BL // HALVES

TRACE = False
LAST_RESULTS = None

# Keras gate order is i,f,g,o (each H wide).  Reorder columns to i,f,o,g so
# sigmoid gates are contiguous.  In the packed z layout blocks are:
# m=0,1 -> i ; m=2,3 -> f ; m=4,5 -> o ; m=6,7 -> g(tanh).
_PERM = np.concatenate(
    [np.arange(0, 2 * H), np.arange(3 * H, 4 * H), np.arange(2 * H, 3 * H)]
)


def _pack_k(w, kt, dt):
    """[kt*128, G] -> [128, kt, G] k-tile packing (partition-major)."""
    return np.ascontiguousarray(
        w.reshape(kt, 128, w.shape[1]).transpose(1, 0, 2)
    ).astype(dt)


def _prep_weights(inputs):
    """Host-side weight prep shared by all cores."""
    f32 = np.float32
    out = {}
    out["emb"] = np.ascontiguousarray(np.asarray(inputs["emb"], f32))
    # g-gate (cols 768:1024 post-perm) scaled by 2 so tanh(z_g) can be
    # computed as 2*sigmoid(2*z_g) - 1 with one fused sigmoid over all gates.
    for nm, kt, dt in [
        ("U1f", 2, RNP), ("U1b", 2, RNP), ("U2f", 2, RNP), ("U2b", 2, RNP),
        ("W2f", 4, BF), ("W2b", 4, BF),
    ]:
        w = np.asarray(inputs[nm], f32)[:, _PERM].copy()
        w[:, 3 * H:] *= 2.0
        out[nm.lower()] = _pack_k(w, kt, dt)
    for nm in ["W1f", "W1b"]:
        w = np.asarray(inputs[nm], f32)[:, _PERM].copy()
        w[:, 3 * H:] *= 2.0
        out[nm.lower()] = np.ascontiguousarray(w).astype(f32)
    for nm in ["b1f", "b1b", "b2f", "b2b"]:
        b = np.asarray(inputs[nm], f32)[_PERM].copy()
        b[3 * H:] *= 2.0
        out[nm.lower()] = np.ascontiguousarray(b.reshape(NM, 128).T).astype(f32)
    wd = np.asarray(inputs["Wd"], f32)  # [2H, C]
    out["wd"] = np.ascontiguousarray(
        wd.reshape(4, 128, C).transpose(1, 0, 2)
    ).astype(BF)
    out["bd"] = np.asarray(inputs["bd"], f32).reshape(1, C).astype(BF)
    return out


def _build():
    """Emit the Tile program (identical SPMD program for every core)."""
    nc = bacc.Bacc("TRN2", target_bir_lowering=False, debug=False,
                   num_devices=NCORES)

    # ---- DRAM I/O ----
    emb_d = nc.dram_tensor("emb", [V, D], F32, kind="ExternalInput")
    xidx_d = nc.dram_tensor("xidx", [128, GCH], I32, kind="ExternalInput")
    wdram = {}
    for nm in ["u1f", "u1b", "u2f", "u2b"]:
        wdram[nm] = nc.dram_tensor(nm, [128, 2, G], RDT, kind="ExternalInput")
    for nm in ["w1f", "w1b"]:
        wdram[nm] = nc.dram_tensor(nm, [128, G], F32, kind="ExternalInput")
    for nm in ["w2f", "w2b"]:
        wdram[nm] = nc.dram_tensor(nm, [128, 4, G], BF16, kind="ExternalInput")
    for nm in ["b1f", "b1b", "b2f", "b2b"]:
        wdram[nm] = nc.dram_tensor(nm, [128, NM], F32, kind="ExternalInput")
    wdram["wd"] = nc.dram_tensor("wd", [128, 4, C], BF16, kind="ExternalInput")
    wdram["bd"] = nc.dram_tensor("bd", [1, C], BF16, kind="ExternalInput")
    out_d = nc.dram_tensor("out", [BL, C], F32, kind="ExternalOutput")

    with tile.TileContext(nc) as tc, \
         tc.tile_pool(name="const", bufs=1) as const, \
         tc.tile_pool(name="work", bufs=2) as work, \
         tc.tile_pool(name="xwp", bufs=2) as xwp, \
         tc.tile_pool(name="psz", bufs=2, space="PSUM") as psz, \
         tc.tile_pool(name="psbig", bufs=2, space="PSUM") as psbig:

        # ---- load weights to SBUF ----
        sb = {}
        for nm, th in wdram.items():
            t_ = const.tile(list(th.shape), th.dtype, name=f"sb_{nm}",
                            tag=f"sb_{nm}")
            nc.sync.dma_start(out=t_[:], in_=th[:])
            sb[nm] = t_
        xidx = const.tile([128, GCH], I32, name="xidx_s", tag="xidx_s")
        nc.sync.dma_start(out=xidx[:], in_=xidx_d[:])

        ident = const.tile([128, 128], F32, name="ident", tag="ident")
        make_identity(nc, ident[:])
        ident_bf = const.tile([128, 128], BF16, name="ident_bf", tag="ident_bf")
        make_identity(nc, ident_bf[:])
        zero_h = const.tile([128, BL], RDT, name="zero_h", tag="zero_h")
        nc.vector.memset(zero_h[:], 0.0)
        ones_r = const.tile([1, BL], BF16, name="ones_r", tag="ones_r")
        nc.vector.memset(ones_r[:], 1.0)

        # big persistent buffers
        eT = const.tile([128, NTOK], F32, name="eT", tag="eT")
        seqT = const.tile([128, 4, NTOK], BF16, name="seqT", tag="seqT")
        c_st = {}
        for dn in ("f", "b"):
            c_st[dn] = const.tile([128, 2 * BL], F32, name=f"c_{dn}",
                                  tag=f"c_{dn}")

        # ---- stage A: embedding gather + transpose -> eT [D, NTOK] f32 ----
        for ch in range(GCH):
            erows = work.tile([128, D], F32, name="erows", tag="erows", bufs=3)
            nc.gpsimd.indirect_dma_start(
                out=erows[:],
                out_offset=None,
                in_=emb_d[:],
                in_offset=bass.IndirectOffsetOnAxis(
                    ap=xidx[:, ch:ch + 1], axis=0),
            )
            tp = psbig.tile([128, 128], F32, name="tp", tag="ps_misc")
            nc.tensor.transpose(out=tp[:], in_=erows[:], identity=ident[:])
            nc.vector.tensor_copy(out=eT[:, ch * 128:(ch + 1) * 128],
                                  in_=tp[:])

        # ---- helpers ----
        def new_xw(dn):
            return xwp.tile([128, NM * CHUNK * BL], BF16, name=f"xw_{dn}",
                            tag=f"xw_{dn}")

        def xw_piece(layer, dn, cc, m, xw):
            """One m-slice of the xW.T+b precompute for chunk cc."""
            cs = slice(cc * CHUNK * BL, (cc + 1) * CHUNK * BL)
            ps = psbig.tile([128, CHUNK * BL], F32, name="ps_xw", tag="ps_xw")
            if layer == 1:
                nc.tensor.matmul(
                    ps[:], lhsT=sb[f"w1{dn}"][:, m * 128:(m + 1) * 128],
                    rhs=eT[:, cs], start=True, stop=True)
            else:
                for k in range(4):
                    nc.tensor.matmul(
                        ps[:],
                        lhsT=sb[f"w2{dn}"][:, k, m * 128:(m + 1) * 128],
                        rhs=seqT[:, k, cs],
                        start=(k == 0), stop=(k == 3))
            nc.scalar.activation(
                out=xw[:, m * CHUNK * BL:(m + 1) * CHUNK * BL],
                in_=ps[:], func=AF.Identity,
                bias=sb[f"b{layer}{dn}"][:, m:m + 1], scale=1.0)

        def xw_chunk(layer, dn, cc):
            xw = new_xw(dn)
            for m in range(NM):
                xw_piece(layer, dn, cc, m, xw)
            return xw

        def scan_pair(layer, steps):
            """One LSTM step for BOTH directions; each direction's batch is
            split into HALVES independent sub-chains (sharing the z PSUM
            bank + xw seed) so more chains overlap the per-step latency.
            steps: list of (dn, t, hp_halves, xw, h_outs, seq_outs)."""
            ctxs = []
            for dn, t, hp_h, xw, h_outs, seq_outs in steps:
                u = sb[f"u{layer}{dn}"]
                z = psz.tile([128, NM * BL], F32, name=f"z_{dn}",
                             tag=f"z_{dn}", bufs=2)
                xw4 = xw.rearrange("p (m s b) -> p m s b", m=NM, s=CHUNK)
                tin = t % CHUNK
                nc.tensor.matmul(z[:], lhsT=ident_bf[:],
                                 rhs=xw4[:, :, tin, :], start=True, stop=False)
                z3 = z[:].rearrange("p (m b) -> p m b", m=NM)
                for hh in range(HALVES):
                    sl = slice(hh * HB, (hh + 1) * HB)
                    for m in range(NM):
                        for k in range(2):
                            nc.tensor.matmul(
                                z3[:, m, sl],
                                lhsT=u[:, k, m * 128:(m + 1) * 128],
                                rhs=hp_h[hh][k], start=False,
                                stop=(hh == HALVES - 1 and m == NM - 1
                                      and k == 1))
                    ctxs.append(dict(dn=dn, z3=z3, sl=sl, hh=hh,
                                     h_out=(h_outs[hh] if h_outs else None),
                                     seq_out=(seq_outs[hh] if seq_outs
                                              else None)))
            for x in ctxs:
                nm_ = f"{x['dn']}{x['hh']}"
                x["g"] = work.tile([128, NM, HB], F32, name="g_" + nm_,
                                   tag=f"g_{nm_}", bufs=3)
                nc.scalar.activation(out=x["g"][:], in_=x["z3"][:, :, x["sl"]],
                                     func=AF.Sigmoid)
            for x in ctxs:
                c3 = c_st[x["dn"]][:].rearrange("p (k b) -> p k b", k=2)
                x["c3"] = c3[:, :, x["sl"]]
                nc.vector.tensor_mul(x["c3"], x["g"][:, 2:4, :], x["c3"])
            for x in ctxs:
                nm_ = f"{x['dn']}{x['hh']}"
                x["gg"] = work.tile([128, 2, HB], F32, name="gg_" + nm_,
                                    tag=f"gg_{nm_}", bufs=3)
                nc.vector.tensor_scalar(out=x["gg"][:],
                                        in0=x["g"][:, 6:8, :],
                                        scalar1=2.0, scalar2=1.0,
                                        op0=mybir.AluOpType.mult,
                                        op1=mybir.AluOpType.subtract)
            for x in ctxs:
                nm_ = f"{x['dn']}{x['hh']}"
                x["tmp"] = work.tile([128, 2, HB], F32, name="tmp_" + nm_,
                                     tag=f"tmp_{nm_}", bufs=3)
                nc.vector.tensor_mul(x["tmp"][:], x["g"][:, 0:2, :],
                                     x["gg"][:])
            for x in ctxs:
                nc.vector.tensor_add(x["c3"], x["c3"], x["tmp"][:])
            for x in ctxs:
                nm_ = f"{x['dn']}{x['hh']}"
                x["th"] = work.tile([128, 2, HB], F32, name="th_" + nm_,
                                    tag=f"th_{nm_}", bufs=3)
                nc.scalar.activation(out=x["th"][:], in_=x["c3"],
                                     func=AF.Tanh)
            for x in ctxs:
                o3 = x["g"][:, 4:6, :]
                if x["h_out"] is not None:
                    nc.vector.tensor_mul(x["h_out"], o3, x["th"][:])
                if x["seq_out"] is not None:
                    nc.vector.tensor_mul(x["seq_out"], o3, x["th"][:])

        # ---- the two BiLSTM phases ----
        hT = {}
        for dn in ("f", "b"):
            hT[dn] = const.tile([128, 2, BL], BF16, name=f"hT_{dn}",
                                tag=f"hT_{dn}")

        def run_phase(layer):
            for dn in ("f", "b"):
                nc.vector.memset(c_st[dn][:], 0.0)
            xw_f = {0: xw_chunk(layer, "f", 0)}
            xw_b = {NCH - 1: xw_chunk(layer, "b", NCH - 1)}
            h = {"f": None, "b": None}
            pieces = []
            for t in range(T):
                if t % CHUNK == 0:
                    # queue next chunks' pieces, spread 1/step below
                    pieces = []
                    cf = t // CHUNK + 1
                    cb = NCH - 2 - t // CHUNK
                    if cf < NCH:
                        xw_f[cf] = new_xw("f")
                        pf = [("f", cf, m, xw_f[cf]) for m in range(NM)]
                    else:
                        pf = []
                    if cb >= 0:
                        xw_b[cb] = new_xw("b")
                        pb = [("b", cb, m, xw_b[cb]) for m in range(NM)]
                    else:
                        pb = []
                    for a, b_ in zip(pf, pb):
                        pieces += [a, b_]
                    pieces += pf[len(pb):] + pb[len(pf):]
                if pieces:
                    dn_, cc_, m_, xwt = pieces.pop(0)
                    xw_piece(layer, dn_, cc_, m_, xwt)
                steps = []
                for dn, tt, xw in (("f", t, xw_f[t // CHUNK]),
                                   ("b", T - 1 - t,
                                    xw_b[(T - 1 - t) // CHUNK])):
                    ks = 0 if dn == "f" else 2
                    hp_h = []
                    for hh in range(HALVES):
                        a, b2 = hh * HB, (hh + 1) * HB
                        if t == 0:
                            hp_h.append([zero_h[:, a:b2], zero_h[:, a:b2]])
                        elif layer == 1 and dn == "f":
                            hp_h.append([seqT[:, k, (tt - 1) * BL + a:
                                              (tt - 1) * BL + b2]
                                         for k in range(2)])
                        elif layer == 1:
                            hp_h.append([seqT[:, 2 + k, (tt + 1) * BL + a:
                                              (tt + 1) * BL + b2]
                                         for k in range(2)])
                        else:
                            hp_h.append([h[dn][hh][:, k, :] for k in range(2)])
                    if layer == 1:
                        seq_outs = [seqT[:, ks:ks + 2, tt * BL + hh * HB:
                                         tt * BL + (hh + 1) * HB]
                                    for hh in range(HALVES)]
                        steps.append((dn, tt, hp_h, xw, None, seq_outs))
                        continue
                    hns = [work.tile([128, 2, HB], RDT, name=f"h2_{dn}{hh}",
                                     tag=f"h2_{dn}{hh}", bufs=3)
                           for hh in range(HALVES)]
                    h_outs = [hn[:, :, :] for hn in hns]
                    seq_outs = None
                    if t == T - 1:
                        seq_outs = [hT[dn][:, :, hh * HB:(hh + 1) * HB]
                                    for hh in range(HALVES)]
                    steps.append((dn, tt, hp_h, xw, h_outs, seq_outs))
                    h[dn] = hns
                scan_pair(layer, steps)

        run_phase(1)
        run_phase(2)

        # ---- dense + softmax ----
        ps = psbig.tile([BL, C], F32, name="ps_d", tag="ps_misc")
        for ki, (dn, k) in enumerate([("f", 0), ("f", 1), ("b", 0), ("b", 1)]):
            nc.tensor.matmul(ps[:], lhsT=hT[dn][:, k, :], rhs=sb["wd"][:, ki, :],
                             start=(ki == 0), stop=False)
        nc.tensor.matmul(ps[:], lhsT=ones_r[:], rhs=sb["bd"][:],
                         start=False, stop=True)
        mx = work.tile([BL, 1], F32, name="mx", tag="mx")
        nc.vector.reduce_max(out=mx[:], in_=ps[:], axis=mybir.AxisListType.X)
        mxn = work.tile([BL, 1], F32, name="mxn", tag="mxn")
        nc.vector.tensor_scalar_mul(mxn[:], mx[:], -1.0)
        ex = work.tile([BL, C], F32, name="ex", tag="ex")
        sm = work.tile([BL, 1], F32, name="sm", tag="sm")
        nc.scalar.activation(out=ex[:], in_=ps[:], func=AF.Exp,
                             bias=mxn[:, 0:1], scale=1.0, accum_out=sm[:])
        rs = work.tile([BL, 1], F32, name="rs", tag="rs")
        nc.vector.reciprocal(rs[:], sm[:])
        osm = work.tile([BL, C], F32, name="osm", tag="osm")
        nc.vector.tensor_scalar_mul(osm[:], ex[:], rs[:, 0:1])
        nc.sync.dma_start(out=out_d[:], in_=osm[:])

    nc.compile()
    return nc


_CACHE = {}


def make_in_maps(inputs):
    w = _prep_weights(inputs)
    x = np.asarray(inputs["x"], np.int32)  # [B, T]
    in_maps = []
    for core in range(NCORES):
        xc = x[core * BL:(core + 1) * BL]            # [BL, T]
        tm = np.ascontiguousarray(xc.T).reshape(-1)  # time-major [T*BL]
        xi = np.ascontiguousarray(tm.reshape(GCH, 128).T).astype(np.int32)
        m = {"xidx": xi}
        m["emb"] = w["emb"]
        for nm in ["u1f", "u1b", "u2f", "u2b", "w1f", "w1b", "w2f", "w2b",
                   "b1f", "b1b", "b2f", "b2b", "wd", "bd"]:
            m[nm] = w[nm]
        in_maps.append(m)
    return in_maps


def get_nc():
    if "nc" not in _CACHE:
        _CACHE["nc"] = _build()
    return _CACHE["nc"]


def kernel(**inputs):
    global LAST_RESULTS
    nc = get_nc()
    in_maps = make_in_maps(inputs)
    res = run_bass_kernel_spmd(nc, in_maps, core_ids=list(range(NCORES)),
                               trace=TRACE)
    LAST_RESULTS = res
    return np.concatenate([r["out"] for r in res.results], axis=0)

